# revision 1
# baseline (speedup 1.0000x reference)
"""EvolveGCN-O kernel for Trainium2 (8 NeuronCores).

Key algebraic restructure: the reference keeps, for node i, only the logits
computed at timestep t_i = time_step[i].  The GCN aggregation at time t is
linear in x, so

  logits_i = cls( relu( (sum_{j->i active@t_i} norm_ji x_j + x_i/deg_i) @ W_{t_i} @ proj^T + b ) )

with norm/deg computed from in-degree counts at t_i.  So instead of 49 full
GCN passes we do ONE edge-aggregation pass (over edges (j,i) with
t_j <= t_i) and one per-timestep-group matmul with P_t = W_t @ proj^T.

Device work per core (nodes sharded by destination, relabeled by (t, core)):
  stage 1: s^T tile accumulation in PSUM via one-hot matmuls
           - self term:   transpose(sw_i * x_i) via identity matmul
           - edge chunks: gather x[src] (indirect DMA), scale by w_e,
                          accumulate y^T @ onehot(dst slot)
  stage 2: z^T = relu(P_t^T s^T + b)   (t static per tile)
  stage 3: lg^T = cls_w^T^T z^T
Host does: GRU weight evolution (tiny FxF chain), degree tables, edge
weights, graph partitioning / relabeling, final unpermute + cls bias.
"""

import ml_dtypes
import numpy as np

N, E, F, H, C, T = 200000, 500000, 166, 128, 2, 49
NCORES = 8
S = 640                      # per-core slots per timestep group (5 tiles)
TILES_PER_T = S // 128       # 5
NT_TILES = T * TILES_PER_T   # 245
NPAD = T * S                 # 31360 slots per core
F1 = 128                     # feature chunk 1
F2 = F - F1                  # 38
PAD_SRC = np.int32(0)  # pad slots gather row 0; onehot weight 0 kills the value

_cache = {}


def _gru_step(Wm, w_ih, w_hh, b_ih, b_hh):
    gi = Wm @ w_ih.T + b_ih
    gh = Wm @ w_hh.T + b_hh
    i_r, i_z, i_n = np.split(gi, 3, axis=-1)
    h_r, h_z, h_n = np.split(gh, 3, axis=-1)
    r = 1.0 / (1.0 + np.exp(-(i_r + h_r)))
    z = 1.0 / (1.0 + np.exp(-(i_z + h_z)))
    nn_ = np.tanh(i_n + r * h_n)
    return (1.0 - z) * nn_ + z * Wm


def _host_prep(x, edge_index, time_step, initial_w, gru_w_ih, gru_w_hh,
               gru_b_ih, gru_b_hh, proj_w, proj_b, cls_w, cls_b):
    src = edge_index[0].astype(np.int64)
    dst = edge_index[1].astype(np.int64)
    t = time_step.astype(np.int64)

    # --- evolve W, fuse with proj ---
    Wm = initial_w.astype(np.float64)
    w_ih = gru_w_ih.astype(np.float64)
    w_hh = gru_w_hh.astype(np.float64)
    b_ih = gru_b_ih.astype(np.float64)
    b_hh = gru_b_hh.astype(np.float64)
    P_stack = np.empty((T, F, H), np.float32)
    projT = proj_w.T.astype(np.float64)
    for step in range(T):
        Wm = _gru_step(Wm, w_ih, w_hh, b_ih, b_hh)
        P_stack[step] = (Wm @ projT).astype(np.float32)

    # --- in-degree table C[v, tau] = #edges (k,v) with t_k <= tau ---
    flat = dst * T + t[src]
    hist = np.bincount(flat, minlength=N * T).astype(np.int32).reshape(N, T)
    Ccum = np.cumsum(hist, axis=1, dtype=np.int32)

    td = t[dst]
    active = t[src] <= td
    deg_dst = Ccum[dst, td] + 1
    deg_src = Ccum[src, td] + 1          # valid where active
    w_e = np.where(active,
                   1.0 / np.sqrt(deg_src.astype(np.float64) * deg_dst.astype(np.float64)),
                   0.0).astype(np.float32)
    sw = (1.0 / (Ccum[np.arange(N), t] + 1.0)).astype(np.float32)  # self weight

    # --- relabel nodes by (t, core, position) ---
    # active in-degree of each node at its own timestep (for tile balancing)
    act_indeg = np.bincount(dst[t[src] <= t[dst]], minlength=N)
    order = np.argsort(t, kind="stable")          # grouped by t
    counts = np.bincount(t, minlength=T)
    starts = np.concatenate(([0], np.cumsum(counts)))[:-1]
    slot_core = np.empty(N, np.int32)
    slot_idx = np.empty(N, np.int32)
    orig_of = np.full((NCORES, NPAD), -1, np.int64)
    for tt in range(T):
        grp = order[starts[tt]: starts[tt] + counts[tt]]
        n_t = counts[tt]
        bounds = (np.arange(NCORES + 1) * n_t) // NCORES
        for c in range(NCORES):
            seg = grp[bounds[c]: bounds[c + 1]]
            k = len(seg)
            assert k <= S, f"t-group {tt} core {c} has {k} > S={S} nodes"
            # ascending-degree packing: low-degree nodes fill early tiles of
            # the group, concentrating edges in the last tiles so most tiles
            # need few (often 0 or 1) 128-edge chunks
            seg = seg[np.argsort(act_indeg[seg], kind="stable")]
            pos2 = np.arange(k)
            slot_core[seg] = c
            slot_idx[seg] = (tt * S + pos2).astype(np.int32)
            orig_of[c, tt * S + pos2] = seg

    # --- per-core relabeled x and self weights ---
    xr_cores, sw_cores = [], []
    for c in range(NCORES):
        ids = orig_of[c]
        valid = ids >= 0
        xr = np.zeros((NPAD, F), np.float32)
        xr[valid] = x[ids[valid]]
        swc = np.zeros(NPAD, np.float32)
        swc[valid] = sw[ids[valid]]
        xr_cores.append(xr)
        sw_cores.append(np.ascontiguousarray(swc.reshape(NT_TILES, 128).T))

    # --- per-core active edge streams sorted by dst slot, chunked per tile ---
    a_idx = np.nonzero(active)[0]
    e_src = src[a_idx]
    e_dst = dst[a_idx]
    e_w = w_e[a_idx]
    e_core = slot_core[e_dst]
    e_slot = slot_idx[e_dst]

    # per-tile-index chunk counts: same across cores (SPMD), variable over ti
    tile_of_edge = e_core.astype(np.int64) * NT_TILES + e_slot // 128
    tile_counts = np.bincount(tile_of_edge, minlength=NCORES * NT_TILES)
    per_ti_max = tile_counts.reshape(NCORES, NT_TILES).max(axis=0)
    klist = np.ceil(per_ti_max / 128).astype(np.int64)   # chunks per tile index
    col_base = np.concatenate(([0], np.cumsum(klist)))   # chunk column base per ti
    ECH = int(col_base[-1])                              # edge chunks per core

    esrcT = np.full((NCORES, 128, ECH), PAD_SRC, np.int32)
    ewT = np.zeros((NCORES, 128, ECH), np.float32)
    elidT = np.zeros((NCORES, 128, ECH), np.float32)
    edge_order = np.lexsort((e_slot, e_core))
    es, ed, ewv, ec, esl = (e_src[edge_order], e_dst[edge_order],
                            e_w[edge_order], e_core[edge_order], e_slot[edge_order])
    tile_sorted = ec.astype(np.int64) * NT_TILES + esl // 128
    # rank of edge within its tile
    tile_start = np.concatenate(([0], np.cumsum(tile_counts)))[:-1]
    rank = np.arange(len(es)) - tile_start[tile_sorted]
    chunk = rank // 128                                  # chunk within tile
    part = rank % 128
    col = col_base[tile_sorted % NT_TILES] + chunk       # chunk column within core
    core_arr = ec
    esrcT[core_arr, part, col] = es.astype(np.int32)
    ewT[core_arr, part, col] = ewv
    elidT[core_arr, part, col] = (esl % 128).astype(np.float32)
    K = tuple(int(v) for v in klist)

    iota_row = np.tile(np.arange(128, dtype=np.float32), (128, 1)).astype(ml_dtypes.bfloat16)
    ident = np.eye(128, dtype=ml_dtypes.bfloat16)
    x_bf = x.astype(ml_dtypes.bfloat16)

    per_core = []
    for c in range(NCORES):
        per_core.append({
            "x": np.ascontiguousarray(x_bf),
            "xr": xr_cores[c].astype(ml_dtypes.bfloat16),
            "swT": sw_cores[c],
            "esrcT": np.ascontiguousarray(esrcT[c]),
            "ewT": np.ascontiguousarray(ewT[c]),
            "elidT": np.ascontiguousarray(elidT[c]),
            "P_stack": P_stack.astype(ml_dtypes.bfloat16),
            "projb": proj_b.reshape(H, 1).astype(np.float32),
            "clsw": cls_w.T.astype(ml_dtypes.bfloat16).copy(),   # [H, C]
            "iota": iota_row,
            "ident": ident,
        })
    return per_core, orig_of, K


def _build(K):
    import concourse.bacc as bacc
    import concourse.bass as bass
    import concourse.mybir as mybir
    import concourse.tile as tile

    klist = list(K)
    col_base = [0]
    for v in klist:
        col_base.append(col_base[-1] + v)
    ECH = col_base[-1]
    nc = bacc.Bacc("TRN2", target_bir_lowering=False, debug=False,
                   num_devices=NCORES)
    dt = mybir.dt.float32
    bf = mybir.dt.bfloat16
    x_d = nc.dram_tensor("x", [N, F], bf, kind="ExternalInput")
    xr_d = nc.dram_tensor("xr", [NPAD, F], bf, kind="ExternalInput")
    swT_d = nc.dram_tensor("swT", [128, NT_TILES], dt, kind="ExternalInput")
    esrcT_d = nc.dram_tensor("esrcT", [128, ECH], mybir.dt.int32, kind="ExternalInput")
    ewT_d = nc.dram_tensor("ewT", [128, ECH], dt, kind="ExternalInput")
    elidT_d = nc.dram_tensor("elidT", [128, ECH], dt, kind="ExternalInput")
    P_d = nc.dram_tensor("P_stack", [T, F, H], bf, kind="ExternalInput")
    projb_d = nc.dram_tensor("projb", [H, 1], dt, kind="ExternalInput")
    clsw_d = nc.dram_tensor("clsw", [H, C], bf, kind="ExternalInput")
    iota_d = nc.dram_tensor("iota", [128, 128], bf, kind="ExternalInput")
    ident_d = nc.dram_tensor("ident", [128, 128], bf, kind="ExternalInput")
    lgT_d = nc.dram_tensor("lgT", [C, NPAD], dt, kind="ExternalOutput")

    with tile.TileContext(nc) as tc:
        with (
            tc.tile_pool(name="const", bufs=1) as cpool,
            tc.tile_pool(name="meta", bufs=1) as mpool,
            tc.tile_pool(name="pt", bufs=2) as ptpool,
            tc.tile_pool(name="xs", bufs=6) as xspool,
            tc.tile_pool(name="y", bufs=20) as ypool,
            tc.tile_pool(name="oh", bufs=12) as ohpool,
            tc.tile_pool(name="st", bufs=2) as stpool,
            tc.tile_pool(name="zt", bufs=2) as ztpool,
            tc.tile_pool(name="lg", bufs=2) as lgpool,
            tc.tile_pool(name="ps", bufs=3, space="PSUM") as pspool,
            tc.tile_pool(name="ps2", bufs=2, space="PSUM") as ps2pool,
            tc.tile_pool(name="pza", bufs=1, space="PSUM") as pzapool,
            tc.tile_pool(name="pzb", bufs=1, space="PSUM") as pzbpool,
            tc.tile_pool(name="pl", bufs=1, space="PSUM") as plpool,
        ):
            iota_sb = cpool.tile([128, 128], bf)
            nc.sync.dma_start(out=iota_sb[:], in_=iota_d[:])
            ident_sb = cpool.tile([128, 128], bf)
            nc.sync.dma_start(out=ident_sb[:], in_=ident_d[:])
            projb_sb = cpool.tile([H, 1], dt)
            nc.sync.dma_start(out=projb_sb[:], in_=projb_d[:])
            clsw_sb = cpool.tile([H, C], bf)
            nc.sync.dma_start(out=clsw_sb[:], in_=clsw_d[:])
            swT_sb = mpool.tile([128, NT_TILES], dt)
            nc.sync.dma_start(out=swT_sb[:], in_=swT_d[:])
            esrcT_sb = mpool.tile([128, ECH], mybir.dt.int32)
            nc.sync.dma_start(out=esrcT_sb[:], in_=esrcT_d[:])
            ewT_sb = mpool.tile([128, ECH], dt)
            nc.sync.dma_start(out=ewT_sb[:], in_=ewT_d[:])
            elidT_sb = mpool.tile([128, ECH], dt)
            nc.sync.dma_start(out=elidT_sb[:], in_=elidT_d[:])

            lg_group = None
            for ti in range(NT_TILES):
                tt = ti // TILES_PER_T
                if ti % TILES_PER_T == 0:
                    pt1 = ptpool.tile([128, H], bf, tag="pt1")
                    nc.sync.dma_start(out=pt1[:], in_=P_d[tt, 0:F1, :])
                    pt2 = ptpool.tile([128, H], bf, tag="pt2")
                    nc.sync.dma_start(out=pt2[0:F2, :], in_=P_d[tt, F1:F, :])

                psum_s = pspool.tile([128, 128], dt, space="PSUM")
                psum_s2 = ps2pool.tile([F2, 128], dt, space="PSUM")
                # ---- self term: psum_s[:,0:128] += (sw*x)^T (chunk1),
                #      psum_s[0:38,128:256] += (sw*x)^T (chunk2)
                xs = xspool.tile([128, F], bf)
                nc.sync.dma_start(out=xs[:], in_=xr_d[ti * 128:(ti + 1) * 128, :])
                kti = klist[ti]
                # self term: out = xs^T @ diag(sw)  (scaled one-hot diagonal)
                dg = ohpool.tile([128, 128], bf, tag="dg")
                nc.vector.tensor_scalar_mul(dg[:], ident_sb[:], swT_sb[:, ti:ti + 1])
                nc.tensor.matmul(out=psum_s[:], lhsT=xs[:, 0:F1],
                                 rhs=dg[:], start=True, stop=kti == 0)
                nc.tensor.matmul(out=psum_s2[:], lhsT=xs[:, F1:F],
                                 rhs=dg[:], start=True, stop=kti == 0)
                # ---- edge chunks: w folded into the one-hot
                for k in range(kti):
                    cidx = col_base[ti] + k
                    last = k == kti - 1
                    y = ypool.tile([128, F], bf, tag="y")
                    nc.gpsimd.indirect_dma_start(
                        out=y[:], out_offset=None, in_=x_d[:],
                        in_offset=bass.IndirectOffsetOnAxis(
                            ap=esrcT_sb[:, cidx:cidx + 1], axis=0),
                    )
                    oh = ohpool.tile([128, 128], bf, tag="oh")
                    nc.vector.tensor_scalar(
                        out=oh[:], in0=iota_sb[:],
                        scalar1=elidT_sb[:, cidx:cidx + 1],
                        scalar2=ewT_sb[:, cidx:cidx + 1],
                        op0=mybir.AluOpType.is_equal,
                        op1=mybir.AluOpType.mult,
                    )
                    nc.tensor.matmul(out=psum_s[:], lhsT=y[:, 0:F1],
                                     rhs=oh[:], start=False, stop=last)
                    nc.tensor.matmul(out=psum_s2[:], lhsT=y[:, F1:F],
                                     rhs=oh[:], start=False, stop=last)
                # ---- sT to SBUF, packed per t-group [128, 640]
                j = ti % TILES_PER_T
                if j == 0:
                    sT1q = stpool.tile([128, S], bf, tag="sT1q")
                    sT2q = stpool.tile([128, S], bf, tag="sT2q")
                nc.vector.tensor_copy(out=sT1q[:, j * 128:(j + 1) * 128], in_=psum_s[:])
                nc.scalar.copy(out=sT2q[0:F2, j * 128:(j + 1) * 128], in_=psum_s2[:])
                if j == TILES_PER_T - 1:
                    # ---- stage 2 batched over the t-group: z^T = relu(P_t^T s^T + b)
                    pz_a = pzapool.tile([128, 512], dt, space="PSUM")
                    pz_b = pzbpool.tile([128, S - 512], dt, space="PSUM")
                    nc.tensor.matmul(out=pz_a[:], lhsT=pt1[:], rhs=sT1q[:, 0:512],
                                     start=True, stop=False)
                    nc.tensor.matmul(out=pz_a[:], lhsT=pt2[0:F2, :],
                                     rhs=sT2q[0:F2, 0:512], start=False, stop=True)
                    nc.tensor.matmul(out=pz_b[:], lhsT=pt1[:], rhs=sT1q[:, 512:S],
                                     start=True, stop=False)
                    nc.tensor.matmul(out=pz_b[:], lhsT=pt2[0:F2, :],
                                     rhs=sT2q[0:F2, 512:S], start=False, stop=True)
                    zTq = ztpool.tile([128, S], bf, tag="zTq")
                    nc.scalar.activation(out=zTq[:, 0:512], in_=pz_a[:],
                                         func=mybir.ActivationFunctionType.Relu,
                                         bias=projb_sb[:, 0:1])
                    nc.scalar.activation(out=zTq[:, 512:S], in_=pz_b[:],
                                         func=mybir.ActivationFunctionType.Relu,
                                         bias=projb_sb[:, 0:1])
                    # ---- stage 3 batched: lg^T for the whole group
                    base = (ti - j) * 128
                    lg = lgpool.tile([C, S], dt, tag="lg")
                    psum_lg = plpool.tile([C, 512], dt, space="PSUM", tag="pl")
                    nc.tensor.matmul(out=psum_lg[:], lhsT=clsw_sb[:],
                                     rhs=zTq[:, 0:512], start=True, stop=True)
                    nc.vector.tensor_copy(out=lg[:, 0:512], in_=psum_lg[:])
                    psum_lg2 = plpool.tile([C, 512], dt, space="PSUM", tag="pl")
                    nc.tensor.matmul(out=psum_lg2[:, 0:S - 512], lhsT=clsw_sb[:],
                                     rhs=zTq[:, 512:S], start=True, stop=True)
                    nc.vector.tensor_copy(out=lg[:, 512:S], in_=psum_lg2[:, 0:S - 512])
                    nc.sync.dma_start(out=lgT_d[:, base:base + S], in_=lg[:])
    nc.compile()
    return nc


def kernel(**inputs):
    from concourse.bass_utils import run_bass_kernel_spmd

    np_inputs = {k: np.asarray(v) for k, v in inputs.items()}
    per_core, orig_of, K = _host_prep(**np_inputs)

    if K not in _cache:
        _cache[K] = _build(K)
    nc = _cache[K]

    res = run_bass_kernel_spmd(nc, per_core, list(range(NCORES)))

    cls_b = np_inputs["cls_b"].astype(np.float32)
    logits = np.zeros((N, C), np.float32)
    for c in range(NCORES):
        ids = orig_of[c]
        valid = ids >= 0
        lgT = res.results[c]["lgT"]                    # [C, NPAD]
        logits[ids[valid]] = lgT.T[valid]
    logits += cls_b
    return logits



# revision 8
# speedup vs baseline: 1.1842x; 1.1842x over previous
"""EvolveGCN-O kernel for Trainium2 (8 NeuronCores).

Key algebraic restructure: the reference keeps, for node i, only the logits
computed at timestep t_i = time_step[i].  The GCN aggregation at time t is
linear in x, so

  logits_i = cls( relu( (sum_{j->i active@t_i} norm_ji x_j + x_i/deg_i) @ W_{t_i} @ proj^T + b ) )

with norm/deg computed from in-degree counts at t_i.  So instead of 49 full
GCN passes we do ONE edge-aggregation pass (over edges (j,i) with
t_j <= t_i) and one per-timestep-group matmul with P_t = W_t @ proj^T.

Sharding (METIS-style partition + halo exchange): nodes are partitioned
across 8 cores by (t, core); each core receives the deduplicated "halo" set
of x rows its edges reference, laid out in first-use order so the edge
aggregation streams it SEQUENTIALLY (no per-row descriptors).  Only repeated
sources (~9% of edges) are fetched by on-device indirect gathers.

Device work per core:
  stage 1: s^T accumulation: psum += slab_chunk^T @ onehot(dst slot, w_e)
           for primary edges; per-group indirect-gathered dup chunks add the
           repeated-source edges; the self term (sw_i * x_i)^T is streamed
           pre-transposed and merged during the PSUM->SBUF copy (DVE add).
  stage 2: z^T = relu(P_t^T s^T + b)   (t static per tile group)
  stage 3: lg^T = cls_w^T^T z^T, stores batched over 7 t-groups
Host does: GRU weight evolution (tiny FxF chain), degree tables, edge
weights, graph partitioning / relabeling / halo tables, unpermute + cls bias.
"""

import ml_dtypes
import numpy as np

N, E, F, H, C, T = 200000, 500000, 166, 128, 2, 49
NCORES = 8
S = 640                      # per-core slots per timestep group (5 tiles)
TILES_PER_T = S // 128       # 5
NT_TILES = T * TILES_PER_T   # 245
NPAD = T * S                 # 31360 slots per core
F1 = 128                     # feature chunk 1
F2 = F - F1                  # 38
GBATCH = 7                   # t-groups per output store

_cache = {}


def _gru_step(Wm, w_ih, w_hh, b_ih, b_hh):
    gi = Wm @ w_ih.T + b_ih
    gh = Wm @ w_hh.T + b_hh
    i_r, i_z, i_n = np.split(gi, 3, axis=-1)
    h_r, h_z, h_n = np.split(gh, 3, axis=-1)
    r = 1.0 / (1.0 + np.exp(-(i_r + h_r)))
    z = 1.0 / (1.0 + np.exp(-(i_z + h_z)))
    nn_ = np.tanh(i_n + r * h_n)
    return (1.0 - z) * nn_ + z * Wm


def _host_prep(x, edge_index, time_step, initial_w, gru_w_ih, gru_w_hh,
               gru_b_ih, gru_b_hh, proj_w, proj_b, cls_w, cls_b):
    src = edge_index[0].astype(np.int64)
    dst = edge_index[1].astype(np.int64)
    t = time_step.astype(np.int64)

    # --- evolve W, fuse with proj ---
    Wm = initial_w.astype(np.float64)
    w_ih = gru_w_ih.astype(np.float64)
    w_hh = gru_w_hh.astype(np.float64)
    b_ih = gru_b_ih.astype(np.float64)
    b_hh = gru_b_hh.astype(np.float64)
    P_stack = np.empty((T, F, H), np.float32)
    projT = proj_w.T.astype(np.float64)
    for step in range(T):
        Wm = _gru_step(Wm, w_ih, w_hh, b_ih, b_hh)
        P_stack[step] = (Wm @ projT).astype(np.float32)

    # --- in-degree table C[v, tau] = #edges (k,v) with t_k <= tau ---
    flat = dst * T + t[src]
    hist = np.bincount(flat, minlength=N * T).astype(np.int32).reshape(N, T)
    Ccum = np.cumsum(hist, axis=1, dtype=np.int32)

    td = t[dst]
    active = t[src] <= td
    deg_dst = Ccum[dst, td] + 1
    deg_src = Ccum[src, td] + 1          # valid where active
    w_e = np.where(active,
                   1.0 / np.sqrt(deg_src.astype(np.float64) * deg_dst.astype(np.float64)),
                   0.0).astype(np.float32)
    sw = (1.0 / (Ccum[np.arange(N), t] + 1.0)).astype(np.float32)  # self weight

    # --- relabel nodes by (t, core, position) ---
    act_indeg = np.bincount(dst[active], minlength=N)
    order = np.argsort(t, kind="stable")          # grouped by t
    counts = np.bincount(t, minlength=T)
    starts = np.concatenate(([0], np.cumsum(counts)))[:-1]
    slot_core = np.empty(N, np.int32)
    slot_idx = np.empty(N, np.int32)
    orig_of = np.full((NCORES, NPAD), -1, np.int64)
    for tt in range(T):
        grp = order[starts[tt]: starts[tt] + counts[tt]]
        n_t = counts[tt]
        bounds = (np.arange(NCORES + 1) * n_t) // NCORES
        for c in range(NCORES):
            seg = grp[bounds[c]: bounds[c + 1]]
            k = len(seg)
            assert k <= S, f"t-group {tt} core {c} has {k} > S={S} nodes"
            # ascending-degree packing: concentrate edges in the last tiles
            seg = seg[np.argsort(act_indeg[seg], kind="stable")]
            pos2 = np.arange(k)
            slot_core[seg] = c
            slot_idx[seg] = (tt * S + pos2).astype(np.int32)
            orig_of[c, tt * S + pos2] = seg

    # --- self rows, pre-scaled + transposed: xrT1 [128, NT*128], xrT2 [38, NT*128]
    xf = x.astype(np.float32)
    xrT1_cores, xrT2_cores = [], []
    for c in range(NCORES):
        ids = orig_of[c]
        valid = ids >= 0
        xr = np.zeros((NPAD, F), np.float32)
        xr[valid] = xf[ids[valid]] * sw[ids[valid]][:, None]
        xr3 = xr.reshape(NT_TILES, 128, F)
        xrT1_cores.append(np.ascontiguousarray(
            xr3[:, :, 0:F1].transpose(2, 0, 1).reshape(F1, NT_TILES * 128)
        ).astype(ml_dtypes.bfloat16))
        xrT2_cores.append(np.ascontiguousarray(
            xr3[:, :, F1:F].transpose(2, 0, 1).reshape(F2, NT_TILES * 128)
        ).astype(ml_dtypes.bfloat16))

    # --- per-core edge streams: split primary (first use of src) vs dup ---
    a_idx = np.nonzero(active)[0]
    e_src_a = src[a_idx]
    e_w_a = w_e[a_idx]
    e_core_a = slot_core[dst[a_idx]]
    e_slot_a = slot_idx[dst[a_idx]]

    x_bf = x.astype(ml_dtypes.bfloat16)
    prim = []            # per core: (src, w, slot) arrays for primary edges
    dups = []            # per core: (src, w, slot) arrays for dup edges
    prim_counts = np.zeros((NCORES, NT_TILES), np.int64)
    for c in range(NCORES):
        m = e_core_a == c
        s_c, w_c, sl_c = e_src_a[m], e_w_a[m], e_slot_a[m]
        o = np.argsort(sl_c, kind="stable")
        s_c, w_c, sl_c = s_c[o], w_c[o], sl_c[o]
        _, first_i = np.unique(s_c, return_index=True)
        is_prim = np.zeros(len(s_c), bool)
        is_prim[first_i] = True
        prim.append((s_c[is_prim], w_c[is_prim], sl_c[is_prim]))
        dups.append((s_c[~is_prim], w_c[~is_prim], sl_c[~is_prim]))
        prim_counts[c] = np.bincount(sl_c[is_prim] // 128, minlength=NT_TILES)

    klist = np.ceil(prim_counts.max(axis=0) / 128).astype(np.int64)
    col_base = np.concatenate(([0], np.cumsum(klist)))
    ECH = int(col_base[-1])

    # primary chunk tables + slab + first-use position of each source
    slab_cores, ewT_cores, elidT_cores = [], [], []
    fpos_cores = []
    for c in range(NCORES):
        s_c, w_c, sl_c = prim[c]
        ti_c = sl_c // 128
        rank = np.arange(len(s_c)) - np.concatenate(
            ([0], np.cumsum(np.bincount(ti_c, minlength=NT_TILES))))[:-1][ti_c]
        cidx = col_base[ti_c] + rank // 128
        part = rank % 128
        slab = np.zeros((128, ECH * F), ml_dtypes.bfloat16)
        slab[part[:, None], (cidx * F)[:, None] + np.arange(F)] = x_bf[s_c]
        ewT = np.zeros((128, ECH), np.float32)
        elidT = np.zeros((128, ECH), np.float32)
        ewT[part, cidx] = w_c
        elidT[part, cidx] = (sl_c % 128).astype(np.float32)
        slab_cores.append(slab)
        ewT_cores.append(ewT)
        elidT_cores.append(elidT)
        # dup gather row index into slab viewed as [128*ECH, F]: p*ECH + cidx
        fpos = dict(zip(s_c.tolist(), (part.astype(np.int64) * ECH + cidx).tolist()))
        fpos_cores.append(fpos)

    # --- dup chunks: per t-group, sorted by slot, chunked by 128 (SPMD-common) ---
    dup_by_gc = [[None] * NCORES for _ in range(T)]
    dg_counts = np.zeros((NCORES, T), np.int64)
    for c in range(NCORES):
        s_c, w_c, sl_c = dups[c]
        g_c = sl_c // S
        o = np.lexsort((sl_c, g_c))
        s_c, w_c, sl_c, g_c = s_c[o], w_c[o], sl_c[o], g_c[o]
        for g in range(T):
            m = g_c == g
            dup_by_gc[g][c] = (s_c[m], w_c[m], sl_c[m])
            dg_counts[c, g] = m.sum()
    DG = dg_counts.max(axis=0)
    dch_group = []       # group of each dup chunk
    dch_spans = []       # tuple of ti values per dup chunk
    for g in range(T):
        for k in range(int(np.ceil(DG[g] / 128))):
            span = set()
            for c in range(NCORES):
                sl = dup_by_gc[g][c][2][k * 128:(k + 1) * 128]
                span.update((sl // 128).tolist())
            if not span:
                continue
            dch_group.append((g, k))
            dch_spans.append(tuple(sorted(int(v) for v in span)))
    NDCH = len(dch_group)
    NSPAN = sum(len(s) for s in dch_spans)

    dupidx_cores = np.zeros((NCORES, 128, max(NDCH, 1)), np.int32)
    ewd_cores = np.zeros((NCORES, 128, max(NSPAN, 1)), np.float32)
    elidd_cores = np.zeros((NCORES, 128, max(NSPAN, 1)), np.float32)
    spi = 0
    for d in range(NDCH):
        g, k = dch_group[d]
        for c in range(NCORES):
            s_c, w_c, sl_c = dup_by_gc[g][c]
            s_k = s_c[k * 128:(k + 1) * 128]
            w_k = w_c[k * 128:(k + 1) * 128]
            sl_k = sl_c[k * 128:(k + 1) * 128]
            nk = len(s_k)
            fp = fpos_cores[c]
            if nk:
                dupidx_cores[c, :nk, d] = [fp[int(sv)] for sv in s_k]
            for si, ti in enumerate(dch_spans[d]):
                mspan = (sl_k // 128) == ti
                col = spi + si
                ewd_cores[c, :nk, col] = np.where(mspan, w_k, 0.0)
                elidd_cores[c, :nk, col] = np.where(mspan, sl_k % 128, 0.0)
        spi += len(dch_spans[d])

    iota_row = np.tile(np.arange(128, dtype=np.float32), (128, 1)).astype(ml_dtypes.bfloat16)
    per_core = []
    for c in range(NCORES):
        per_core.append({
            "slab": slab_cores[c],
            "xrT1": xrT1_cores[c],
            "xrT2": xrT2_cores[c],
            "ewT": ewT_cores[c],
            "elidT": elidT_cores[c],
            "dupidx": dupidx_cores[c],
            "ewd": ewd_cores[c],
            "elidd": elidd_cores[c],
            "P_stack": np.ascontiguousarray(
                P_stack.transpose(1, 0, 2).reshape(F, T * H)).astype(ml_dtypes.bfloat16),
            "projb": proj_b.reshape(H, 1).astype(np.float32),
            "clsw": cls_w.T.astype(ml_dtypes.bfloat16).copy(),   # [H, C]
            "iota": iota_row,
        })
    K = (tuple(int(v) for v in klist), tuple(dch_spans))
    return per_core, orig_of, K


def _build(K):
    import concourse.bacc as bacc
    import concourse.bass as bass
    import concourse.mybir as mybir
    import concourse.tile as tile

    klist, dch_spans = K
    klist = list(klist)
    col_base = [0]
    for v in klist:
        col_base.append(col_base[-1] + v)
    ECH = col_base[-1]
    NDCH = len(dch_spans)
    NSPAN = sum(len(s) for s in dch_spans)
    ti_spans = [[] for _ in range(NT_TILES)]
    spi = 0
    for d in range(NDCH):
        for si, ti in enumerate(dch_spans[d]):
            ti_spans[ti].append((d, spi + si))
        spi += len(dch_spans[d])

    nc = bacc.Bacc("TRN2", target_bir_lowering=False, debug=False,
                   num_devices=NCORES)
    dt = mybir.dt.float32
    bf = mybir.dt.bfloat16
    slab_d = nc.dram_tensor("slab", [128, ECH * F], bf, kind="ExternalInput")
    xrT1_d = nc.dram_tensor("xrT1", [F1, NT_TILES * 128], bf, kind="ExternalInput")
    xrT2_d = nc.dram_tensor("xrT2", [F2, NT_TILES * 128], bf, kind="ExternalInput")
    ewT_d = nc.dram_tensor("ewT", [128, ECH], dt, kind="ExternalInput")
    elidT_d = nc.dram_tensor("elidT", [128, ECH], dt, kind="ExternalInput")
    dupidx_d = nc.dram_tensor("dupidx", [128, max(NDCH, 1)], mybir.dt.int32,
                              kind="ExternalInput")
    ewd_d = nc.dram_tensor("ewd", [128, max(NSPAN, 1)], dt, kind="ExternalInput")
    elidd_d = nc.dram_tensor("elidd", [128, max(NSPAN, 1)], dt, kind="ExternalInput")
    P_d = nc.dram_tensor("P_stack", [F, T * H], bf, kind="ExternalInput")
    projb_d = nc.dram_tensor("projb", [H, 1], dt, kind="ExternalInput")
    clsw_d = nc.dram_tensor("clsw", [H, C], bf, kind="ExternalInput")
    iota_d = nc.dram_tensor("iota", [128, 128], bf, kind="ExternalInput")
    lgT_d = nc.dram_tensor("lgT", [C, NPAD], dt, kind="ExternalOutput")

    NSLAB = 8
    SW = (ECH + NSLAB - 1) // NSLAB          # chunks per slab piece
    XSLAB = 5
    XW = NT_TILES // XSLAB                   # tiles per xrT piece

    with tile.TileContext(nc) as tc:
        with (
            tc.tile_pool(name="const", bufs=1) as cpool,
            tc.tile_pool(name="meta", bufs=1) as mpool,
            tc.tile_pool(name="slab", bufs=3) as slabpool,
            tc.tile_pool(name="xrt", bufs=2) as xrtpool,
            tc.tile_pool(name="yd", bufs=1) as ydpool,
            tc.tile_pool(name="oh", bufs=12) as ohpool,
            tc.tile_pool(name="st", bufs=2) as stpool,
            tc.tile_pool(name="zt", bufs=2) as ztpool,
            tc.tile_pool(name="lg", bufs=2) as lgpool,
            tc.tile_pool(name="ps", bufs=3, space="PSUM") as pspool,
            tc.tile_pool(name="ps2", bufs=2, space="PSUM") as ps2pool,
            tc.tile_pool(name="pza", bufs=1, space="PSUM") as pzapool,
            tc.tile_pool(name="pzb", bufs=1, space="PSUM") as pzbpool,
            tc.tile_pool(name="pl", bufs=1, space="PSUM") as plpool,
        ):
            iota_sb = cpool.tile([128, 128], bf)
            nc.sync.dma_start(out=iota_sb[:], in_=iota_d[:])
            projb_sb = cpool.tile([H, 1], dt)
            nc.sync.dma_start(out=projb_sb[:], in_=projb_d[:])
            clsw_sb = cpool.tile([H, C], bf)
            nc.sync.dma_start(out=clsw_sb[:], in_=clsw_d[:])
            ewT_sb = mpool.tile([128, ECH], dt)
            nc.sync.dma_start(out=ewT_sb[:], in_=ewT_d[:])
            elidT_sb = mpool.tile([128, ECH], dt)
            nc.sync.dma_start(out=elidT_sb[:], in_=elidT_d[:])
            dupidx_sb = mpool.tile([128, max(NDCH, 1)], mybir.dt.int32)
            nc.sync.dma_start(out=dupidx_sb[:], in_=dupidx_d[:])
            ewd_sb = mpool.tile([128, max(NSPAN, 1)], dt)
            nc.sync.dma_start(out=ewd_sb[:], in_=ewd_d[:])
            elidd_sb = mpool.tile([128, max(NSPAN, 1)], dt)
            nc.sync.dma_start(out=elidd_sb[:], in_=elidd_d[:])
            P1_sb = mpool.tile([F1, T * H], bf)
            nc.sync.dma_start(out=P1_sb[:], in_=P_d[0:F1, :])
            P2_sb = mpool.tile([F2, T * H], bf)
            nc.sync.dma_start(out=P2_sb[:], in_=P_d[F1:F, :])

            # dup-chunk gathers (from the DRAM slab, row view [128*ECH, F])
            slab_rows = slab_d[:].rearrange("p (c f) -> (p c) f", f=F)
            ydup = []
            for d in range(NDCH):
                y = ydpool.tile([128, F], bf, tag=f"yd{d}")
                nc.gpsimd.indirect_dma_start(
                    out=y[:], out_offset=None, in_=slab_rows,
                    in_offset=bass.IndirectOffsetOnAxis(
                        ap=dupidx_sb[:, d:d + 1], axis=0),
                )
                ydup.append(y)

            slabs = []
            for sB in range(NSLAB):
                c0 = sB * SW
                w = min(SW, ECH - c0)
                stile = slabpool.tile([128, SW * F], bf, tag="slab")
                nc.sync.dma_start(out=stile[:, 0:w * F],
                                  in_=slab_d[:, c0 * F:(c0 + w) * F])
                slabs.append(stile)

            xrt1s, xrt2s = [], []
            for sB in range(XSLAB):
                c0 = sB * XW * 128
                x1 = xrtpool.tile([F1, XW * 128], bf, tag="xrt1")
                nc.sync.dma_start(out=x1[:], in_=xrT1_d[:, c0:c0 + XW * 128])
                x2 = xrtpool.tile([F2, XW * 128], bf, tag="xrt2")
                nc.sync.dma_start(out=x2[:], in_=xrT2_d[:, c0:c0 + XW * 128])
                xrt1s.append(x1)
                xrt2s.append(x2)

            for ti in range(NT_TILES):
                tt = ti // TILES_PER_T
                j = ti % TILES_PER_T
                kti = klist[ti]
                spans = ti_spans[ti]
                nmm = kti + len(spans)
                x1 = xrt1s[ti // XW]
                x2 = xrt2s[ti // XW]
                xo = (ti % XW) * 128
                if j == 0:
                    sT1q = stpool.tile([128, S], bf, tag="sT1q")
                    sT2q = stpool.tile([128, S], bf, tag="sT2q")
                if nmm == 0:
                    nc.vector.tensor_copy(out=sT1q[:, j * 128:(j + 1) * 128],
                                          in_=x1[:, xo:xo + 128])
                    nc.scalar.copy(out=sT2q[0:F2, j * 128:(j + 1) * 128],
                                   in_=x2[:, xo:xo + 128])
                else:
                    psum_s = pspool.tile([128, 128], dt, space="PSUM")
                    psum_s2 = ps2pool.tile([F2, 128], dt, space="PSUM")
                    first = True
                    for k in range(kti):
                        cidx = col_base[ti] + k
                        last = k == kti - 1 and not spans
                        ysl = slabs[cidx // SW]
                        off = (cidx % SW) * F
                        oh = ohpool.tile([128, 128], bf, tag="oh")
                        nc.vector.tensor_scalar(
                            out=oh[:], in0=iota_sb[:],
                            scalar1=elidT_sb[:, cidx:cidx + 1],
                            scalar2=ewT_sb[:, cidx:cidx + 1],
                            op0=mybir.AluOpType.is_equal,
                            op1=mybir.AluOpType.mult,
                        )
                        nc.tensor.matmul(out=psum_s[:], lhsT=ysl[:, off:off + F1],
                                         rhs=oh[:], start=first, stop=last)
                        nc.tensor.matmul(out=psum_s2[:], lhsT=ysl[:, off + F1:off + F],
                                         rhs=oh[:], start=first, stop=last)
                        first = False
                    for si, (d, spcol) in enumerate(spans):
                        last = si == len(spans) - 1
                        ohd = ohpool.tile([128, 128], bf, tag="oh")
                        nc.vector.tensor_scalar(
                            out=ohd[:], in0=iota_sb[:],
                            scalar1=elidd_sb[:, spcol:spcol + 1],
                            scalar2=ewd_sb[:, spcol:spcol + 1],
                            op0=mybir.AluOpType.is_equal,
                            op1=mybir.AluOpType.mult,
                        )
                        yD = ydup[d]
                        nc.tensor.matmul(out=psum_s[:], lhsT=yD[:, 0:F1],
                                         rhs=ohd[:], start=first, stop=last)
                        nc.tensor.matmul(out=psum_s2[:], lhsT=yD[:, F1:F],
                                         rhs=ohd[:], start=first, stop=last)
                        first = False
                    # merge self term during PSUM -> SBUF copy
                    nc.vector.tensor_tensor(
                        out=sT1q[:, j * 128:(j + 1) * 128], in0=psum_s[:],
                        in1=x1[:, xo:xo + 128], op=mybir.AluOpType.add)
                    nc.vector.tensor_tensor(
                        out=sT2q[0:F2, j * 128:(j + 1) * 128], in0=psum_s2[:],
                        in1=x2[:, xo:xo + 128], op=mybir.AluOpType.add)
                if j == TILES_PER_T - 1:
                    # ---- stage 2 batched: z^T = relu(P_t^T s^T + b)
                    pz_a = pzapool.tile([128, 512], dt, space="PSUM")
                    pz_b = pzbpool.tile([128, S - 512], dt, space="PSUM")
                    pc0 = tt * H
                    nc.tensor.matmul(out=pz_a[:], lhsT=P1_sb[:, pc0:pc0 + H],
                                     rhs=sT1q[:, 0:512], start=True, stop=False)
                    nc.tensor.matmul(out=pz_a[:], lhsT=P2_sb[:, pc0:pc0 + H],
                                     rhs=sT2q[0:F2, 0:512], start=False, stop=True)
                    nc.tensor.matmul(out=pz_b[:], lhsT=P1_sb[:, pc0:pc0 + H],
                                     rhs=sT1q[:, 512:S], start=True, stop=False)
                    nc.tensor.matmul(out=pz_b[:], lhsT=P2_sb[:, pc0:pc0 + H],
                                     rhs=sT2q[0:F2, 512:S], start=False, stop=True)
                    zTq = ztpool.tile([128, S], bf, tag="zTq")
                    nc.scalar.activation(out=zTq[:, 0:512], in_=pz_a[:],
                                         func=mybir.ActivationFunctionType.Relu,
                                         bias=projb_sb[:, 0:1])
                    nc.scalar.activation(out=zTq[:, 512:S], in_=pz_b[:],
                                         func=mybir.ActivationFunctionType.Relu,
                                         bias=projb_sb[:, 0:1])
                    # ---- stage 3 batched: lg^T for the whole group
                    if tt % GBATCH == 0:
                        lgb = lgpool.tile([C, GBATCH * S], dt, tag="lgb")
                    lo = (tt % GBATCH) * S
                    psum_lg = plpool.tile([C, 512], dt, space="PSUM", tag="pl")
                    nc.tensor.matmul(out=psum_lg[:], lhsT=clsw_sb[:],
                                     rhs=zTq[:, 0:512], start=True, stop=True)
                    nc.vector.tensor_copy(out=lgb[:, lo:lo + 512], in_=psum_lg[:])
                    psum_lg2 = plpool.tile([C, 512], dt, space="PSUM", tag="pl")
                    nc.tensor.matmul(out=psum_lg2[:, 0:S - 512], lhsT=clsw_sb[:],
                                     rhs=zTq[:, 512:S], start=True, stop=True)
                    nc.vector.tensor_copy(out=lgb[:, lo + 512:lo + S],
                                          in_=psum_lg2[:, 0:S - 512])
                    if tt % GBATCH == GBATCH - 1 or tt == T - 1:
                        b0 = (tt - tt % GBATCH) * S
                        nc.sync.dma_start(
                            out=lgT_d[:, b0:b0 + (tt % GBATCH) * S + S],
                            in_=lgb[:, 0:(tt % GBATCH) * S + S])
    nc.compile()
    return nc


def kernel(**inputs):
    from concourse.bass_utils import run_bass_kernel_spmd

    np_inputs = {k: np.asarray(v) for k, v in inputs.items()}
    per_core, orig_of, K = _host_prep(**np_inputs)

    if K not in _cache:
        _cache[K] = _build(K)
    nc = _cache[K]

    res = run_bass_kernel_spmd(nc, per_core, list(range(NCORES)))

    cls_b = np_inputs["cls_b"].astype(np.float32)
    logits = np.zeros((N, C), np.float32)
    for c in range(NCORES):
        ids = orig_of[c]
        valid = ids >= 0
        lgT = res.results[c]["lgT"]                    # [C, NPAD]
        logits[ids[valid]] = lgT.T[valid]
    logits += cls_b
    return logits


# revision 11
# speedup vs baseline: 1.3491x; 1.1393x over previous
"""EvolveGCN-O kernel for Trainium2 (8 NeuronCores).

Key algebraic restructure: the reference keeps, for node i, only the logits
computed at timestep t_i = time_step[i].  The GCN aggregation at time t is
linear in x, so

  logits_i = cls( relu( (sum_{j->i active@t_i} norm_ji x_j + x_i/deg_i) @ W_{t_i} @ proj^T + b ) )

with norm/deg computed from in-degree counts at t_i.  So instead of 49 full
GCN passes we do ONE edge-aggregation pass (over edges (j,i) with
t_j <= t_i) and one per-timestep-group matmul with P_t = W_t @ proj^T.

Sharding (METIS-style partition + halo exchange): nodes are partitioned
across 8 cores by (t, core); each core receives the deduplicated "halo" set
of x rows its edges reference, laid out in first-use order so the edge
aggregation streams it SEQUENTIALLY (no per-row descriptors).  Only repeated
sources (~9% of edges) are fetched by on-device indirect gathers.

Device work per core:
  stage 1: s^T accumulation: psum += slab_chunk^T @ onehot(dst slot, w_e)
           for primary edges; per-group indirect-gathered dup chunks add the
           repeated-source edges; the self term (sw_i * x_i)^T is streamed
           pre-transposed and merged during the PSUM->SBUF copy (DVE add).
  stage 2: z^T = relu(P_t^T s^T + b)   (t static per tile group)
  stage 3: lg^T = cls_w^T^T z^T, stores batched over 7 t-groups
Host does: GRU weight evolution (tiny FxF chain), degree tables, edge
weights, graph partitioning / relabeling / halo tables, unpermute + cls bias.
"""

import ml_dtypes
import numpy as np

N, E, F, H, C, T = 200000, 500000, 166, 128, 2, 49
NCORES = 8
S = 640                      # per-core slots per timestep group (5 tiles)
TILES_PER_T = S // 128       # 5
NT_TILES = T * TILES_PER_T   # 245
NPAD = T * S                 # 31360 slots per core
F1 = 128                     # feature chunk 1
F2 = F - F1                  # 38
GBATCH = 7                   # t-groups per output store

_cache = {}


def _gru_step(Wm, w_ih, w_hh, b_ih, b_hh):
    gi = Wm @ w_ih.T + b_ih
    gh = Wm @ w_hh.T + b_hh
    i_r, i_z, i_n = np.split(gi, 3, axis=-1)
    h_r, h_z, h_n = np.split(gh, 3, axis=-1)
    r = 1.0 / (1.0 + np.exp(-(i_r + h_r)))
    z = 1.0 / (1.0 + np.exp(-(i_z + h_z)))
    nn_ = np.tanh(i_n + r * h_n)
    return (1.0 - z) * nn_ + z * Wm


def _host_prep(x, edge_index, time_step, initial_w, gru_w_ih, gru_w_hh,
               gru_b_ih, gru_b_hh, proj_w, proj_b, cls_w, cls_b):
    src = edge_index[0].astype(np.int64)
    dst = edge_index[1].astype(np.int64)
    t = time_step.astype(np.int64)

    # --- evolve W, fuse with proj ---
    Wm = initial_w.astype(np.float64)
    w_ih = gru_w_ih.astype(np.float64)
    w_hh = gru_w_hh.astype(np.float64)
    b_ih = gru_b_ih.astype(np.float64)
    b_hh = gru_b_hh.astype(np.float64)
    P_stack = np.empty((T, F, H), np.float32)
    projT = proj_w.T.astype(np.float64)
    for step in range(T):
        Wm = _gru_step(Wm, w_ih, w_hh, b_ih, b_hh)
        P_stack[step] = (Wm @ projT).astype(np.float32)

    # --- in-degree table C[v, tau] = #edges (k,v) with t_k <= tau ---
    flat = dst * T + t[src]
    hist = np.bincount(flat, minlength=N * T).astype(np.int32).reshape(N, T)
    Ccum = np.cumsum(hist, axis=1, dtype=np.int32)

    td = t[dst]
    active = t[src] <= td
    deg_dst = Ccum[dst, td] + 1
    deg_src = Ccum[src, td] + 1          # valid where active
    w_e = np.where(active,
                   1.0 / np.sqrt(deg_src.astype(np.float64) * deg_dst.astype(np.float64)),
                   0.0).astype(np.float32)
    sw = (1.0 / (Ccum[np.arange(N), t] + 1.0)).astype(np.float32)  # self weight

    # --- relabel nodes by (t, core, position) ---
    act_indeg = np.bincount(dst[active], minlength=N)
    order = np.argsort(t, kind="stable")          # grouped by t
    counts = np.bincount(t, minlength=T)
    starts = np.concatenate(([0], np.cumsum(counts)))[:-1]
    slot_core = np.empty(N, np.int32)
    slot_idx = np.empty(N, np.int32)
    orig_of = np.full((NCORES, NPAD), -1, np.int64)
    for tt in range(T):
        grp = order[starts[tt]: starts[tt] + counts[tt]]
        n_t = counts[tt]
        bounds = (np.arange(NCORES + 1) * n_t) // NCORES
        for c in range(NCORES):
            seg = grp[bounds[c]: bounds[c + 1]]
            k = len(seg)
            assert k <= S, f"t-group {tt} core {c} has {k} > S={S} nodes"
            # ascending-degree packing: concentrate edges in the last tiles
            seg = seg[np.argsort(act_indeg[seg], kind="stable")]
            pos2 = np.arange(k)
            slot_core[seg] = c
            slot_idx[seg] = (tt * S + pos2).astype(np.int32)
            orig_of[c, tt * S + pos2] = seg

    # --- self rows, pre-scaled + transposed: xrT1 [128, NT*128], xrT2 [38, NT*128]
    xf = x.astype(np.float32)
    xrT1_cores, xrT2_cores = [], []
    for c in range(NCORES):
        ids = orig_of[c]
        valid = ids >= 0
        xr = np.zeros((NPAD, F), np.float32)
        xr[valid] = xf[ids[valid]] * sw[ids[valid]][:, None]
        xr3 = xr.reshape(NT_TILES, 128, F)
        xrT1_cores.append(np.ascontiguousarray(
            xr3[:, :, 0:F1].transpose(2, 0, 1).reshape(F1, NT_TILES * 128)
        ).astype(ml_dtypes.bfloat16))
        xrT2_cores.append(np.ascontiguousarray(
            xr3[:, :, F1:F].transpose(2, 0, 1).reshape(F2, NT_TILES * 128)
        ).astype(ml_dtypes.bfloat16))

    # --- per-core edge streams: split primary (first use of src) vs dup ---
    a_idx = np.nonzero(active)[0]
    e_src_a = src[a_idx]
    e_w_a = w_e[a_idx]
    e_core_a = slot_core[dst[a_idx]]
    e_slot_a = slot_idx[dst[a_idx]]

    x_bf = x.astype(ml_dtypes.bfloat16)
    prim = []            # per core: (src, w, slot) arrays for primary edges
    dups = []            # per core: (src, w, slot) arrays for dup edges
    prim_counts = np.zeros((NCORES, NT_TILES), np.int64)
    for c in range(NCORES):
        m = e_core_a == c
        s_c, w_c, sl_c = e_src_a[m], e_w_a[m], e_slot_a[m]
        o = np.argsort(sl_c, kind="stable")
        s_c, w_c, sl_c = s_c[o], w_c[o], sl_c[o]
        _, first_i = np.unique(s_c, return_index=True)
        is_prim = np.zeros(len(s_c), bool)
        is_prim[first_i] = True
        prim.append((s_c[is_prim], w_c[is_prim], sl_c[is_prim]))
        dups.append((s_c[~is_prim], w_c[~is_prim], sl_c[~is_prim]))
        prim_counts[c] = np.bincount(sl_c[is_prim] // 128, minlength=NT_TILES)

    klist = np.ceil(prim_counts.max(axis=0) / 128).astype(np.int64)
    col_base = np.concatenate(([0], np.cumsum(klist)))
    ECH = int(col_base[-1])

    # primary chunk tables + slab + first-use position of each source
    slab_cores, ewT_cores, elidT_cores = [], [], []
    fpos_cores = []
    for c in range(NCORES):
        s_c, w_c, sl_c = prim[c]
        ti_c = sl_c // 128
        rank = np.arange(len(s_c)) - np.concatenate(
            ([0], np.cumsum(np.bincount(ti_c, minlength=NT_TILES))))[:-1][ti_c]
        cidx = col_base[ti_c] + rank // 128
        part = rank % 128
        slab = np.zeros((128, ECH * F), ml_dtypes.bfloat16)
        slab[part[:, None], (cidx * F)[:, None] + np.arange(F)] = x_bf[s_c]
        ewT = np.zeros((128, ECH), np.float32)
        elidT = np.zeros((128, ECH), np.int64)
        ewT[part, cidx] = w_c
        elidT[part, cidx] = sl_c % 128
        slab_cores.append(slab)
        ewT_cores.append(ewT)
        elidT_cores.append(elidT)
        # dup gather row index into slab viewed as [128*ECH, F]: p*ECH + cidx
        fpos = dict(zip(s_c.tolist(), (part.astype(np.int64) * ECH + cidx).tolist()))
        fpos_cores.append(fpos)

    # --- dup chunks: per t-group, sorted by slot, chunked by 128 (SPMD-common) ---
    dup_by_gc = [[None] * NCORES for _ in range(T)]
    dg_counts = np.zeros((NCORES, T), np.int64)
    for c in range(NCORES):
        s_c, w_c, sl_c = dups[c]
        g_c = sl_c // S
        o = np.lexsort((sl_c, g_c))
        s_c, w_c, sl_c, g_c = s_c[o], w_c[o], sl_c[o], g_c[o]
        for g in range(T):
            m = g_c == g
            dup_by_gc[g][c] = (s_c[m], w_c[m], sl_c[m])
            dg_counts[c, g] = m.sum()
    DG = dg_counts.max(axis=0)
    dch_group = []       # group of each dup chunk
    dch_spans = []       # tuple of ti values per dup chunk
    for g in range(T):
        for k in range(int(np.ceil(DG[g] / 128))):
            span = set()
            for c in range(NCORES):
                sl = dup_by_gc[g][c][2][k * 128:(k + 1) * 128]
                span.update((sl // 128).tolist())
            if not span:
                continue
            dch_group.append((g, k))
            dch_spans.append(tuple(sorted(int(v) for v in span)))
    NDCH = len(dch_group)
    NSPAN = sum(len(s) for s in dch_spans)

    dupidx_cores = np.zeros((NCORES, 128, max(NDCH, 1)), np.int32)
    ewd_cores = np.zeros((NCORES, 128, max(NSPAN, 1)), np.float32)
    elidd_cores = np.zeros((NCORES, 128, max(NSPAN, 1)), np.int64)
    spi = 0
    for d in range(NDCH):
        g, k = dch_group[d]
        for c in range(NCORES):
            s_c, w_c, sl_c = dup_by_gc[g][c]
            s_k = s_c[k * 128:(k + 1) * 128]
            w_k = w_c[k * 128:(k + 1) * 128]
            sl_k = sl_c[k * 128:(k + 1) * 128]
            nk = len(s_k)
            fp = fpos_cores[c]
            if nk:
                dupidx_cores[c, :nk, d] = [fp[int(sv)] for sv in s_k]
            for si, ti in enumerate(dch_spans[d]):
                mspan = (sl_k // 128) == ti
                col = spi + si
                ewd_cores[c, :nk, col] = np.where(mspan, w_k, 0.0)
                elidd_cores[c, :nk, col] = np.where(mspan, sl_k % 128, 0)
        spi += len(dch_spans[d])

    # ---- pack dense one-hot stream in device consumption order ----
    ti_spans_h = [[] for _ in range(NT_TILES)]
    spi = 0
    for d in range(NDCH):
        for si, ti in enumerate(dch_spans[d]):
            ti_spans_h[ti].append(spi + si)
        spi += len(dch_spans[d])
    order_cols = []          # (kind, col): kind 0 = primary cidx, 1 = span col
    col_base_h = np.concatenate(([0], np.cumsum(klist)))
    for ti in range(NT_TILES):
        for k in range(int(klist[ti])):
            order_cols.append((0, int(col_base_h[ti]) + k))
        for spcol in ti_spans_h[ti]:
            order_cols.append((1, spcol))
    NOH = len(order_cols)
    oh_cores = []
    rows = np.arange(128)
    for c in range(NCORES):
        ohs = np.zeros((128, NOH * 128), ml_dtypes.bfloat16)
        for q, (kind, col) in enumerate(order_cols):
            if kind == 0:
                wv, lid = ewT_cores[c][:, col], elidT_cores[c][:, col]
            else:
                wv, lid = ewd_cores[c][:, col], elidd_cores[c][:, col]
            ohs[rows, q * 128 + lid] = wv
        oh_cores.append(ohs)

    per_core = []
    for c in range(NCORES):
        per_core.append({
            "slab": slab_cores[c],
            "xrT1": xrT1_cores[c],
            "xrT2": xrT2_cores[c],
            "ohs": oh_cores[c],
            "dupidx": dupidx_cores[c],
            "P_stack": np.ascontiguousarray(
                P_stack.transpose(1, 0, 2).reshape(F, T * H)).astype(ml_dtypes.bfloat16),
            "projb": proj_b.reshape(H, 1).astype(np.float32),
            "clsw": cls_w.T.astype(ml_dtypes.bfloat16).copy(),   # [H, C]
        })
    K = (tuple(int(v) for v in klist), tuple(dch_spans))
    return per_core, orig_of, K


def _build(K):
    import concourse.bacc as bacc
    import concourse.bass as bass
    import concourse.mybir as mybir
    import concourse.tile as tile

    klist, dch_spans = K
    klist = list(klist)
    col_base = [0]
    for v in klist:
        col_base.append(col_base[-1] + v)
    ECH = col_base[-1]
    NDCH = len(dch_spans)
    NSPAN = sum(len(s) for s in dch_spans)
    ti_spans = [[] for _ in range(NT_TILES)]
    spi = 0
    for d in range(NDCH):
        for si, ti in enumerate(dch_spans[d]):
            ti_spans[ti].append((d, spi + si))
        spi += len(dch_spans[d])
    # oh stream slot of each (kind, col), mirroring host packing order
    oh_of = {}
    q = 0
    for ti in range(NT_TILES):
        for k in range(klist[ti]):
            oh_of[(0, col_base[ti] + k)] = q
            q += 1
        for d, spcol in ti_spans[ti]:
            oh_of[(1, spcol)] = q
            q += 1

    nc = bacc.Bacc("TRN2", target_bir_lowering=False, debug=False,
                   num_devices=NCORES)
    dt = mybir.dt.float32
    bf = mybir.dt.bfloat16
    slab_d = nc.dram_tensor("slab", [128, ECH * F], bf, kind="ExternalInput")
    xrT1_d = nc.dram_tensor("xrT1", [F1, NT_TILES * 128], bf, kind="ExternalInput")
    xrT2_d = nc.dram_tensor("xrT2", [F2, NT_TILES * 128], bf, kind="ExternalInput")
    NOH = ECH + NSPAN
    ohs_d = nc.dram_tensor("ohs", [128, NOH * 128], bf, kind="ExternalInput")
    dupidx_d = nc.dram_tensor("dupidx", [128, max(NDCH, 1)], mybir.dt.int32,
                              kind="ExternalInput")
    P_d = nc.dram_tensor("P_stack", [F, T * H], bf, kind="ExternalInput")
    projb_d = nc.dram_tensor("projb", [H, 1], dt, kind="ExternalInput")
    clsw_d = nc.dram_tensor("clsw", [H, C], bf, kind="ExternalInput")
    lgT_d = nc.dram_tensor("lgT", [C, NPAD], dt, kind="ExternalOutput")

    NSLAB = 8
    SW = (ECH + NSLAB - 1) // NSLAB          # chunks per slab piece
    XSLAB = 5
    XW = NT_TILES // XSLAB                   # tiles per xrT piece

    with tile.TileContext(nc) as tc:
        with (
            tc.tile_pool(name="const", bufs=1) as cpool,
            tc.tile_pool(name="meta", bufs=1) as mpool,
            tc.tile_pool(name="slab", bufs=3) as slabpool,
            tc.tile_pool(name="ohslab", bufs=2) as ohslabpool,
            tc.tile_pool(name="xrt", bufs=2) as xrtpool,
            tc.tile_pool(name="yd", bufs=1) as ydpool,
            tc.tile_pool(name="oh", bufs=12) as ohpool,
            tc.tile_pool(name="st", bufs=2) as stpool,
            tc.tile_pool(name="zt", bufs=2) as ztpool,
            tc.tile_pool(name="lg", bufs=1) as lgpool,
            tc.tile_pool(name="ps", bufs=3, space="PSUM") as pspool,
            tc.tile_pool(name="ps2", bufs=2, space="PSUM") as ps2pool,
            tc.tile_pool(name="pza", bufs=1, space="PSUM") as pzapool,
            tc.tile_pool(name="pzb", bufs=1, space="PSUM") as pzbpool,
            tc.tile_pool(name="pl", bufs=1, space="PSUM") as plpool,
        ):
            projb_sb = cpool.tile([H, 1], dt)
            nc.sync.dma_start(out=projb_sb[:], in_=projb_d[:])
            clsw_sb = cpool.tile([H, C], bf)
            nc.sync.dma_start(out=clsw_sb[:], in_=clsw_d[:])
            dupidx_sb = mpool.tile([128, max(NDCH, 1)], mybir.dt.int32)
            nc.sync.dma_start(out=dupidx_sb[:], in_=dupidx_d[:])
            P1_sb = mpool.tile([F1, T * H], bf)
            nc.sync.dma_start(out=P1_sb[:], in_=P_d[0:F1, :])
            P2_sb = mpool.tile([F2, T * H], bf)
            nc.sync.dma_start(out=P2_sb[:], in_=P_d[F1:F, :])

            # dup-chunk gathers (from the DRAM slab, row view [128*ECH, F])
            slab_rows = slab_d[:].rearrange("p (c f) -> (p c) f", f=F)
            ydup = []
            for d in range(NDCH):
                y = ydpool.tile([128, F], bf, tag=f"yd{d}")
                nc.gpsimd.indirect_dma_start(
                    out=y[:], out_offset=None, in_=slab_rows,
                    in_offset=bass.IndirectOffsetOnAxis(
                        ap=dupidx_sb[:, d:d + 1], axis=0),
                )
                ydup.append(y)

            NOSLAB = 8
            OW = (NOH + NOSLAB - 1) // NOSLAB
            ohslabs = []
            for sB in range(NOSLAB):
                c0 = sB * OW
                w = min(OW, NOH - c0)
                otile = ohslabpool.tile([128, OW * 128], bf, tag="ohslab")
                nc.sync.dma_start(out=otile[:, 0:w * 128],
                                  in_=ohs_d[:, c0 * 128:(c0 + w) * 128])
                ohslabs.append(otile)

            slabs = []
            for sB in range(NSLAB):
                c0 = sB * SW
                w = min(SW, ECH - c0)
                stile = slabpool.tile([128, SW * F], bf, tag="slab")
                nc.sync.dma_start(out=stile[:, 0:w * F],
                                  in_=slab_d[:, c0 * F:(c0 + w) * F])
                slabs.append(stile)

            xrt1s, xrt2s = [], []
            for sB in range(XSLAB):
                c0 = sB * XW * 128
                x1 = xrtpool.tile([F1, XW * 128], bf, tag="xrt1")
                nc.sync.dma_start(out=x1[:], in_=xrT1_d[:, c0:c0 + XW * 128])
                x2 = xrtpool.tile([F2, XW * 128], bf, tag="xrt2")
                nc.sync.dma_start(out=x2[:], in_=xrT2_d[:, c0:c0 + XW * 128])
                xrt1s.append(x1)
                xrt2s.append(x2)

            for ti in range(NT_TILES):
                tt = ti // TILES_PER_T
                j = ti % TILES_PER_T
                kti = klist[ti]
                spans = ti_spans[ti]
                nmm = kti + len(spans)
                x1 = xrt1s[ti // XW]
                x2 = xrt2s[ti // XW]
                xo = (ti % XW) * 128
                if j == 0:
                    sT1q = stpool.tile([128, S], bf, tag="sT1q")
                    sT2q = stpool.tile([128, S], bf, tag="sT2q")
                if nmm == 0:
                    nc.vector.tensor_copy(out=sT1q[:, j * 128:(j + 1) * 128],
                                          in_=x1[:, xo:xo + 128])
                    nc.scalar.copy(out=sT2q[0:F2, j * 128:(j + 1) * 128],
                                   in_=x2[:, xo:xo + 128])
                else:
                    psum_s = pspool.tile([128, 128], dt, space="PSUM")
                    psum_s2 = ps2pool.tile([F2, 128], dt, space="PSUM")
                    # ops: (lhsT source, oh stream slot) per chunk, F1 pass then F2
                    ops = []
                    for k in range(kti):
                        cidx = col_base[ti] + k
                        ysl = slabs[cidx // SW]
                        off = (cidx % SW) * F
                        q = oh_of[(0, cidx)]
                        ops.append((ysl, off, q))
                    for d, spcol in spans:
                        ops.append((ydup[d], 0, oh_of[(1, spcol)]))
                    for i, (ysl, off, q) in enumerate(ops):
                        ohsl = ohslabs[q // OW]
                        oho = (q % OW) * 128
                        nc.tensor.matmul(out=psum_s[:], lhsT=ysl[:, off:off + F1],
                                         rhs=ohsl[:, oho:oho + 128],
                                         start=i == 0, stop=i == len(ops) - 1)
                    for i, (ysl, off, q) in enumerate(ops):
                        ohsl = ohslabs[q // OW]
                        oho = (q % OW) * 128
                        nc.tensor.matmul(out=psum_s2[:], lhsT=ysl[:, off + F1:off + F],
                                         rhs=ohsl[:, oho:oho + 128],
                                         start=i == 0, stop=i == len(ops) - 1)
                    # merge self term during PSUM -> SBUF copy
                    nc.vector.tensor_tensor(
                        out=sT1q[:, j * 128:(j + 1) * 128], in0=psum_s[:],
                        in1=x1[:, xo:xo + 128], op=mybir.AluOpType.add)
                    nc.vector.tensor_tensor(
                        out=sT2q[0:F2, j * 128:(j + 1) * 128], in0=psum_s2[:],
                        in1=x2[:, xo:xo + 128], op=mybir.AluOpType.add)
                if j == TILES_PER_T - 1:
                    # ---- stage 2 batched: z^T = relu(P_t^T s^T + b)
                    pz_a = pzapool.tile([128, 512], dt, space="PSUM")
                    pz_b = pzbpool.tile([128, S - 512], dt, space="PSUM")
                    pc0 = tt * H
                    nc.tensor.matmul(out=pz_a[:], lhsT=P1_sb[:, pc0:pc0 + H],
                                     rhs=sT1q[:, 0:512], start=True, stop=False)
                    nc.tensor.matmul(out=pz_b[:], lhsT=P1_sb[:, pc0:pc0 + H],
                                     rhs=sT1q[:, 512:S], start=True, stop=False)
                    nc.tensor.matmul(out=pz_a[:], lhsT=P2_sb[:, pc0:pc0 + H],
                                     rhs=sT2q[0:F2, 0:512], start=False, stop=True)
                    nc.tensor.matmul(out=pz_b[:], lhsT=P2_sb[:, pc0:pc0 + H],
                                     rhs=sT2q[0:F2, 512:S], start=False, stop=True)
                    zTq = ztpool.tile([128, S], bf, tag="zTq")
                    nc.scalar.activation(out=zTq[:, 0:512], in_=pz_a[:],
                                         func=mybir.ActivationFunctionType.Relu,
                                         bias=projb_sb[:, 0:1])
                    nc.scalar.activation(out=zTq[:, 512:S], in_=pz_b[:],
                                         func=mybir.ActivationFunctionType.Relu,
                                         bias=projb_sb[:, 0:1])
                    # ---- stage 3 batched: lg^T for the whole group
                    if tt % GBATCH == 0:
                        lgb = lgpool.tile([C, GBATCH * S], dt, tag="lgb")
                    lo = (tt % GBATCH) * S
                    psum_lg = plpool.tile([C, 512], dt, space="PSUM", tag="pl")
                    nc.tensor.matmul(out=psum_lg[:], lhsT=clsw_sb[:],
                                     rhs=zTq[:, 0:512], start=True, stop=True)
                    nc.scalar.copy(out=lgb[:, lo:lo + 512], in_=psum_lg[:])
                    psum_lg2 = plpool.tile([C, 512], dt, space="PSUM", tag="pl")
                    nc.tensor.matmul(out=psum_lg2[:, 0:S - 512], lhsT=clsw_sb[:],
                                     rhs=zTq[:, 512:S], start=True, stop=True)
                    nc.scalar.copy(out=lgb[:, lo + 512:lo + S],
                                   in_=psum_lg2[:, 0:S - 512])
                    if tt % GBATCH == GBATCH - 1 or tt == T - 1:
                        b0 = (tt - tt % GBATCH) * S
                        nc.sync.dma_start(
                            out=lgT_d[:, b0:b0 + (tt % GBATCH) * S + S],
                            in_=lgb[:, 0:(tt % GBATCH) * S + S])
    nc.compile()
    return nc


def kernel(**inputs):
    from concourse.bass_utils import run_bass_kernel_spmd

    np_inputs = {k: np.asarray(v) for k, v in inputs.items()}
    per_core, orig_of, K = _host_prep(**np_inputs)

    if K not in _cache:
        _cache[K] = _build(K)
    nc = _cache[K]

    res = run_bass_kernel_spmd(nc, per_core, list(range(NCORES)))

    cls_b = np_inputs["cls_b"].astype(np.float32)
    logits = np.zeros((N, C), np.float32)
    for c in range(NCORES):
        ids = orig_of[c]
        valid = ids >= 0
        lgT = res.results[c]["lgT"]                    # [C, NPAD]
        logits[ids[valid]] = lgT.T[valid]
    logits += cls_b
    return logits


# revision 12
# speedup vs baseline: 1.4608x; 1.0828x over previous
"""EvolveGCN-O kernel for Trainium2 (8 NeuronCores).

Key algebraic restructure: the reference keeps, for node i, only the logits
computed at timestep t_i = time_step[i].  The GCN aggregation at time t is
linear in x, so

  logits_i = cls( relu( (sum_{j->i active@t_i} norm_ji x_j + x_i/deg_i) @ W_{t_i} @ proj^T + b ) )

with norm/deg computed from in-degree counts at t_i.  So instead of 49 full
GCN passes we do ONE edge-aggregation pass (over edges (j,i) with
t_j <= t_i) and one per-timestep-group matmul with P_t = W_t @ proj^T.

Sharding (METIS-style partition + halo exchange): nodes are partitioned
across 8 cores by (t, core); each core receives the deduplicated "halo" set
of x rows its edges reference, laid out in first-use order so the edge
aggregation streams it SEQUENTIALLY (no per-row descriptors).  Only repeated
sources (~9% of edges) are fetched by on-device indirect gathers.

Device work per core:
  stage 1: s^T accumulation: psum += slab_chunk^T @ onehot(dst slot, w_e)
           for primary edges; per-group indirect-gathered dup chunks add the
           repeated-source edges; the self term (sw_i * x_i)^T is streamed
           pre-transposed and merged during the PSUM->SBUF copy (DVE add).
  stage 2: z^T = relu(P_t^T s^T + b)   (t static per tile group)
  stage 3: lg^T = cls_w^T^T z^T, stores batched over 7 t-groups
Host does: GRU weight evolution (tiny FxF chain), degree tables, edge
weights, graph partitioning / relabeling / halo tables, unpermute + cls bias.
"""

import ml_dtypes
import numpy as np

N, E, F, H, C, T = 200000, 500000, 166, 128, 2, 49
NCORES = 8
S = 640                      # per-core slots per timestep group (5 tiles)
TILES_PER_T = S // 128       # 5
NT_TILES = T * TILES_PER_T   # 245
NPAD = T * S                 # 31360 slots per core
F1 = 128                     # feature chunk 1
F2 = F - F1                  # 38
GBATCH = 7                   # t-groups per output store

_cache = {}


def _gru_step(Wm, w_ih, w_hh, b_ih, b_hh):
    gi = Wm @ w_ih.T + b_ih
    gh = Wm @ w_hh.T + b_hh
    i_r, i_z, i_n = np.split(gi, 3, axis=-1)
    h_r, h_z, h_n = np.split(gh, 3, axis=-1)
    r = 1.0 / (1.0 + np.exp(-(i_r + h_r)))
    z = 1.0 / (1.0 + np.exp(-(i_z + h_z)))
    nn_ = np.tanh(i_n + r * h_n)
    return (1.0 - z) * nn_ + z * Wm


def _host_prep(x, edge_index, time_step, initial_w, gru_w_ih, gru_w_hh,
               gru_b_ih, gru_b_hh, proj_w, proj_b, cls_w, cls_b):
    src = edge_index[0].astype(np.int64)
    dst = edge_index[1].astype(np.int64)
    t = time_step.astype(np.int64)

    # --- evolve W, fuse with proj ---
    Wm = initial_w.astype(np.float64)
    w_ih = gru_w_ih.astype(np.float64)
    w_hh = gru_w_hh.astype(np.float64)
    b_ih = gru_b_ih.astype(np.float64)
    b_hh = gru_b_hh.astype(np.float64)
    P_stack = np.empty((T, F, H), np.float32)
    projT = proj_w.T.astype(np.float64)
    for step in range(T):
        Wm = _gru_step(Wm, w_ih, w_hh, b_ih, b_hh)
        P_stack[step] = (Wm @ projT).astype(np.float32)

    # --- in-degree table C[v, tau] = #edges (k,v) with t_k <= tau ---
    flat = dst * T + t[src]
    hist = np.bincount(flat, minlength=N * T).astype(np.int32).reshape(N, T)
    Ccum = np.cumsum(hist, axis=1, dtype=np.int32)

    td = t[dst]
    active = t[src] <= td
    deg_dst = Ccum[dst, td] + 1
    deg_src = Ccum[src, td] + 1          # valid where active
    w_e = np.where(active,
                   1.0 / np.sqrt(deg_src.astype(np.float64) * deg_dst.astype(np.float64)),
                   0.0).astype(np.float32)
    sw = (1.0 / (Ccum[np.arange(N), t] + 1.0)).astype(np.float32)  # self weight

    # --- relabel nodes by (t, core, position) ---
    act_indeg = np.bincount(dst[active], minlength=N)
    order = np.argsort(t, kind="stable")          # grouped by t
    counts = np.bincount(t, minlength=T)
    starts = np.concatenate(([0], np.cumsum(counts)))[:-1]
    slot_core = np.empty(N, np.int32)
    slot_idx = np.empty(N, np.int32)
    orig_of = np.full((NCORES, NPAD), -1, np.int64)
    for tt in range(T):
        grp = order[starts[tt]: starts[tt] + counts[tt]]
        n_t = counts[tt]
        bounds = (np.arange(NCORES + 1) * n_t) // NCORES
        for c in range(NCORES):
            seg = grp[bounds[c]: bounds[c + 1]]
            k = len(seg)
            assert k <= S, f"t-group {tt} core {c} has {k} > S={S} nodes"
            # ascending-degree packing: concentrate edges in the last tiles
            seg = seg[np.argsort(act_indeg[seg], kind="stable")]
            pos2 = np.arange(k)
            slot_core[seg] = c
            slot_idx[seg] = (tt * S + pos2).astype(np.int32)
            orig_of[c, tt * S + pos2] = seg

    # --- self rows, pre-scaled + transposed: xrT1 [128, NT*128], xrT2 [38, NT*128]
    xf = x.astype(np.float32)
    xrT1_cores, xrT2_cores = [], []
    for c in range(NCORES):
        ids = orig_of[c]
        valid = ids >= 0
        xr = np.zeros((NPAD, F), np.float32)
        xr[valid] = xf[ids[valid]] * sw[ids[valid]][:, None]
        xr3 = xr.reshape(NT_TILES, 128, F)
        xrT1_cores.append(np.ascontiguousarray(
            xr3[:, :, 0:F1].transpose(2, 0, 1).reshape(F1, NT_TILES * 128)
        ).astype(ml_dtypes.bfloat16))
        xrT2_cores.append(np.ascontiguousarray(
            xr3[:, :, F1:F].transpose(2, 0, 1).reshape(F2, NT_TILES * 128)
        ).astype(ml_dtypes.bfloat16))

    # --- per-core edge streams: split primary (first use of src) vs dup ---
    a_idx = np.nonzero(active)[0]
    e_src_a = src[a_idx]
    e_w_a = w_e[a_idx]
    e_core_a = slot_core[dst[a_idx]]
    e_slot_a = slot_idx[dst[a_idx]]

    x_bf = x.astype(ml_dtypes.bfloat16)
    prim = []            # per core: (src, w, slot) arrays for primary edges
    dups = []            # per core: (src, w, slot) arrays for dup edges
    prim_counts = np.zeros((NCORES, NT_TILES), np.int64)
    for c in range(NCORES):
        m = e_core_a == c
        s_c, w_c, sl_c = e_src_a[m], e_w_a[m], e_slot_a[m]
        o = np.argsort(sl_c, kind="stable")
        s_c, w_c, sl_c = s_c[o], w_c[o], sl_c[o]
        _, first_i = np.unique(s_c, return_index=True)
        is_prim = np.zeros(len(s_c), bool)
        is_prim[first_i] = True
        prim.append((s_c[is_prim], w_c[is_prim], sl_c[is_prim]))
        dups.append((s_c[~is_prim], w_c[~is_prim], sl_c[~is_prim]))
        prim_counts[c] = np.bincount(sl_c[is_prim] // 128, minlength=NT_TILES)

    klist = np.ceil(prim_counts.max(axis=0) / 128).astype(np.int64)
    col_base = np.concatenate(([0], np.cumsum(klist)))
    ECH = int(col_base[-1])

    # primary chunk tables + slab + first-use position of each source
    slab_cores, ewT_cores, elidT_cores = [], [], []
    fpos_cores = []
    for c in range(NCORES):
        s_c, w_c, sl_c = prim[c]
        ti_c = sl_c // 128
        rank = np.arange(len(s_c)) - np.concatenate(
            ([0], np.cumsum(np.bincount(ti_c, minlength=NT_TILES))))[:-1][ti_c]
        cidx = col_base[ti_c] + rank // 128
        part = rank % 128
        slab = np.zeros((128, ECH * F), ml_dtypes.bfloat16)
        slab[part[:, None], (cidx * F)[:, None] + np.arange(F)] = x_bf[s_c]
        ewT = np.zeros((128, ECH), np.float32)
        elidT = np.zeros((128, ECH), np.int64)
        ewT[part, cidx] = w_c
        elidT[part, cidx] = sl_c % 128
        slab_cores.append(slab)
        ewT_cores.append(ewT)
        elidT_cores.append(elidT)
        # dup gather row index into slab viewed as [128*ECH, F]: p*ECH + cidx
        fpos = dict(zip(s_c.tolist(), (part.astype(np.int64) * ECH + cidx).tolist()))
        fpos_cores.append(fpos)

    # --- dup chunks: per t-group, sorted by slot, chunked by 128 (SPMD-common) ---
    dup_by_gc = [[None] * NCORES for _ in range(T)]
    dg_counts = np.zeros((NCORES, T), np.int64)
    for c in range(NCORES):
        s_c, w_c, sl_c = dups[c]
        g_c = sl_c // S
        o = np.lexsort((sl_c, g_c))
        s_c, w_c, sl_c, g_c = s_c[o], w_c[o], sl_c[o], g_c[o]
        for g in range(T):
            m = g_c == g
            dup_by_gc[g][c] = (s_c[m], w_c[m], sl_c[m])
            dg_counts[c, g] = m.sum()
    DG = dg_counts.max(axis=0)
    dch_group = []       # group of each dup chunk
    dch_spans = []       # tuple of ti values per dup chunk
    for g in range(T):
        for k in range(int(np.ceil(DG[g] / 128))):
            span = set()
            for c in range(NCORES):
                sl = dup_by_gc[g][c][2][k * 128:(k + 1) * 128]
                span.update((sl // 128).tolist())
            if not span:
                continue
            dch_group.append((g, k))
            dch_spans.append(tuple(sorted(int(v) for v in span)))
    NDCH = len(dch_group)
    NSPAN = sum(len(s) for s in dch_spans)

    dupidx_cores = np.zeros((NCORES, 128, max(NDCH, 1)), np.int32)
    ewd_cores = np.zeros((NCORES, 128, max(NSPAN, 1)), np.float32)
    elidd_cores = np.zeros((NCORES, 128, max(NSPAN, 1)), np.int64)
    spi = 0
    for d in range(NDCH):
        g, k = dch_group[d]
        for c in range(NCORES):
            s_c, w_c, sl_c = dup_by_gc[g][c]
            s_k = s_c[k * 128:(k + 1) * 128]
            w_k = w_c[k * 128:(k + 1) * 128]
            sl_k = sl_c[k * 128:(k + 1) * 128]
            nk = len(s_k)
            fp = fpos_cores[c]
            if nk:
                dupidx_cores[c, :nk, d] = [fp[int(sv)] for sv in s_k]
            for si, ti in enumerate(dch_spans[d]):
                mspan = (sl_k // 128) == ti
                col = spi + si
                ewd_cores[c, :nk, col] = np.where(mspan, w_k, 0.0)
                elidd_cores[c, :nk, col] = np.where(mspan, sl_k % 128, 0)
        spi += len(dch_spans[d])

    # ---- pack dense one-hot stream in device consumption order ----
    ti_spans_h = [[] for _ in range(NT_TILES)]
    spi = 0
    for d in range(NDCH):
        for si, ti in enumerate(dch_spans[d]):
            ti_spans_h[ti].append(spi + si)
        spi += len(dch_spans[d])
    order_cols = []          # (kind, col): kind 0 = primary cidx, 1 = span col
    col_base_h = np.concatenate(([0], np.cumsum(klist)))
    for ti in range(NT_TILES):
        for k in range(int(klist[ti])):
            order_cols.append((0, int(col_base_h[ti]) + k))
        for spcol in ti_spans_h[ti]:
            order_cols.append((1, spcol))
    NOH = len(order_cols)
    oh_cores = []
    rows = np.arange(128)
    for c in range(NCORES):
        ohs = np.zeros((128, NOH * 128), ml_dtypes.bfloat16)
        for q, (kind, col) in enumerate(order_cols):
            if kind == 0:
                wv, lid = ewT_cores[c][:, col], elidT_cores[c][:, col]
            else:
                wv, lid = ewd_cores[c][:, col], elidd_cores[c][:, col]
            ohs[rows, q * 128 + lid] = wv
        oh_cores.append(ohs)

    per_core = []
    for c in range(NCORES):
        per_core.append({
            "slab": slab_cores[c],
            "xrT1": xrT1_cores[c],
            "xrT2": xrT2_cores[c],
            "ohs": oh_cores[c],
            "dupidx": dupidx_cores[c],
            "P_stack": np.ascontiguousarray(
                P_stack.transpose(1, 0, 2).reshape(F, T * H)).astype(ml_dtypes.bfloat16),
            "projb": proj_b.reshape(H, 1).astype(np.float32),
            "clsw": cls_w.T.astype(ml_dtypes.bfloat16).copy(),   # [H, C]
        })
    K = (tuple(int(v) for v in klist), tuple(dch_spans))
    return per_core, orig_of, K


def _build(K):
    import concourse.bacc as bacc
    import concourse.bass as bass
    import concourse.mybir as mybir
    import concourse.tile as tile

    klist, dch_spans = K
    klist = list(klist)
    col_base = [0]
    for v in klist:
        col_base.append(col_base[-1] + v)
    ECH = col_base[-1]
    NDCH = len(dch_spans)
    NSPAN = sum(len(s) for s in dch_spans)
    ti_spans = [[] for _ in range(NT_TILES)]
    spi = 0
    for d in range(NDCH):
        for si, ti in enumerate(dch_spans[d]):
            ti_spans[ti].append((d, spi + si))
        spi += len(dch_spans[d])
    # oh stream slot of each (kind, col), mirroring host packing order
    oh_of = {}
    q = 0
    for ti in range(NT_TILES):
        for k in range(klist[ti]):
            oh_of[(0, col_base[ti] + k)] = q
            q += 1
        for d, spcol in ti_spans[ti]:
            oh_of[(1, spcol)] = q
            q += 1

    nc = bacc.Bacc("TRN2", target_bir_lowering=False, debug=False,
                   num_devices=NCORES)
    dt = mybir.dt.float32
    bf = mybir.dt.bfloat16
    slab_d = nc.dram_tensor("slab", [128, ECH * F], bf, kind="ExternalInput")
    xrT1_d = nc.dram_tensor("xrT1", [F1, NT_TILES * 128], bf, kind="ExternalInput")
    xrT2_d = nc.dram_tensor("xrT2", [F2, NT_TILES * 128], bf, kind="ExternalInput")
    NOH = ECH + NSPAN
    ohs_d = nc.dram_tensor("ohs", [128, NOH * 128], bf, kind="ExternalInput")
    dupidx_d = nc.dram_tensor("dupidx", [128, max(NDCH, 1)], mybir.dt.int32,
                              kind="ExternalInput")
    P_d = nc.dram_tensor("P_stack", [F, T * H], bf, kind="ExternalInput")
    projb_d = nc.dram_tensor("projb", [H, 1], dt, kind="ExternalInput")
    clsw_d = nc.dram_tensor("clsw", [H, C], bf, kind="ExternalInput")
    lgT_d = nc.dram_tensor("lgT", [C, NPAD], dt, kind="ExternalOutput")

    def pieces(total, first, big):
        bounds = [0]
        while bounds[-1] < total:
            step = first if len(bounds) <= 4 else big
            bounds.append(min(total, bounds[-1] + step))
        return bounds

    slab_b = pieces(ECH, 8, 48)
    piece_of_chunk = [0] * ECH
    for pi in range(len(slab_b) - 1):
        for cdx in range(slab_b[pi], slab_b[pi + 1]):
            piece_of_chunk[cdx] = pi
    xrt_b = pieces(NT_TILES, 10, 50)
    piece_of_tile = [0] * NT_TILES
    for pi in range(len(xrt_b) - 1):
        for tix in range(xrt_b[pi], xrt_b[pi + 1]):
            piece_of_tile[tix] = pi

    with tile.TileContext(nc) as tc:
        with (
            tc.tile_pool(name="const", bufs=1) as cpool,
            tc.tile_pool(name="meta", bufs=1) as mpool,
            tc.tile_pool(name="slab", bufs=3) as slabpool,
            tc.tile_pool(name="ohslab", bufs=2) as ohslabpool,
            tc.tile_pool(name="xrt", bufs=2) as xrtpool,
            tc.tile_pool(name="yd", bufs=1) as ydpool,
            tc.tile_pool(name="oh", bufs=12) as ohpool,
            tc.tile_pool(name="st", bufs=2) as stpool,
            tc.tile_pool(name="zt", bufs=2) as ztpool,
            tc.tile_pool(name="lg", bufs=1) as lgpool,
            tc.tile_pool(name="ps", bufs=4, space="PSUM") as pspool,
            tc.tile_pool(name="ps2", bufs=2, space="PSUM") as ps2pool,
            tc.tile_pool(name="pz", bufs=2, space="PSUM") as pzpool,
        ):
            projb_sb = cpool.tile([H, 1], dt)
            nc.sync.dma_start(out=projb_sb[:], in_=projb_d[:])
            clsw_sb = cpool.tile([H, C], bf)
            nc.sync.dma_start(out=clsw_sb[:], in_=clsw_d[:])
            dupidx_sb = mpool.tile([128, max(NDCH, 1)], mybir.dt.int32)
            nc.sync.dma_start(out=dupidx_sb[:], in_=dupidx_d[:])
            P1_sb = mpool.tile([F1, T * H], bf)
            nc.sync.dma_start(out=P1_sb[:], in_=P_d[0:F1, :])
            P2_sb = mpool.tile([F2, T * H], bf)
            nc.sync.dma_start(out=P2_sb[:], in_=P_d[F1:F, :])

            # dup-chunk gathers (from the DRAM slab, row view [128*ECH, F])
            slab_rows = slab_d[:].rearrange("p (c f) -> (p c) f", f=F)
            ydup = []
            for d in range(NDCH):
                y = ydpool.tile([128, F], bf, tag=f"yd{d}")
                nc.gpsimd.indirect_dma_start(
                    out=y[:], out_offset=None, in_=slab_rows,
                    in_offset=bass.IndirectOffsetOnAxis(
                        ap=dupidx_sb[:, d:d + 1], axis=0),
                )
                ydup.append(y)

            oh_b = pieces(NOH, 10, 72)
            piece_of_oh = [0] * NOH
            for pi in range(len(oh_b) - 1):
                for qx in range(oh_b[pi], oh_b[pi + 1]):
                    piece_of_oh[qx] = pi
            OHW = max(oh_b[i + 1] - oh_b[i] for i in range(len(oh_b) - 1))
            SLW = max(slab_b[i + 1] - slab_b[i] for i in range(len(slab_b) - 1))
            XRW = max(xrt_b[i + 1] - xrt_b[i] for i in range(len(xrt_b) - 1))

            # interleave initial pieces so tile 0's inputs land first
            ohslabs, slabs, xrt1s, xrt2s = [], [], [], []
            nmax = max(len(oh_b), len(slab_b), len(xrt_b)) - 1
            for pi in range(nmax):
                if pi < len(slab_b) - 1:
                    c0, c1 = slab_b[pi], slab_b[pi + 1]
                    stile = slabpool.tile([128, SLW * F], bf, tag="slab")
                    nc.sync.dma_start(out=stile[:, 0:(c1 - c0) * F],
                                      in_=slab_d[:, c0 * F:c1 * F])
                    slabs.append(stile)
                if pi < len(oh_b) - 1:
                    c0, c1 = oh_b[pi], oh_b[pi + 1]
                    otile = ohslabpool.tile([128, OHW * 128], bf, tag="ohslab")
                    nc.sync.dma_start(out=otile[:, 0:(c1 - c0) * 128],
                                      in_=ohs_d[:, c0 * 128:c1 * 128])
                    ohslabs.append(otile)
                if pi < len(xrt_b) - 1:
                    c0, c1 = xrt_b[pi], xrt_b[pi + 1]
                    x1 = xrtpool.tile([F1, XRW * 128], bf, tag="xrt1")
                    nc.sync.dma_start(out=x1[:, 0:(c1 - c0) * 128],
                                      in_=xrT1_d[:, c0 * 128:c1 * 128])
                    x2 = xrtpool.tile([F2, XRW * 128], bf, tag="xrt2")
                    nc.sync.dma_start(out=x2[:, 0:(c1 - c0) * 128],
                                      in_=xrT2_d[:, c0 * 128:c1 * 128])
                    xrt1s.append(x1)
                    xrt2s.append(x2)

            for ti in range(NT_TILES):
                tt = ti // TILES_PER_T
                j = ti % TILES_PER_T
                kti = klist[ti]
                spans = ti_spans[ti]
                nmm = kti + len(spans)
                xpi = piece_of_tile[ti]
                x1 = xrt1s[xpi]
                x2 = xrt2s[xpi]
                xo = (ti - xrt_b[xpi]) * 128
                if j == 0:
                    sT1q = stpool.tile([128, S], bf, tag="sT1q")
                    sT2q = stpool.tile([128, S], bf, tag="sT2q")
                if nmm == 0:
                    nc.vector.tensor_copy(out=sT1q[:, j * 128:(j + 1) * 128],
                                          in_=x1[:, xo:xo + 128])
                    nc.scalar.copy(out=sT2q[0:F2, j * 128:(j + 1) * 128],
                                   in_=x2[:, xo:xo + 128])
                else:
                    psum_s = pspool.tile([128, 128], dt, space="PSUM")
                    psum_s2 = ps2pool.tile([F2, 128], dt, space="PSUM")
                    # ops: (lhsT source, oh stream slot) per chunk, F1 pass then F2
                    ops = []
                    for k in range(kti):
                        cidx = col_base[ti] + k
                        spi_ = piece_of_chunk[cidx]
                        ysl = slabs[spi_]
                        off = (cidx - slab_b[spi_]) * F
                        q = oh_of[(0, cidx)]
                        ops.append((ysl, off, q))
                    for d, spcol in spans:
                        ops.append((ydup[d], 0, oh_of[(1, spcol)]))
                    for i, (ysl, off, q) in enumerate(ops):
                        opi = piece_of_oh[q]
                        ohsl = ohslabs[opi]
                        oho = (q - oh_b[opi]) * 128
                        nc.tensor.matmul(out=psum_s[:], lhsT=ysl[:, off:off + F1],
                                         rhs=ohsl[:, oho:oho + 128],
                                         start=i == 0, stop=i == len(ops) - 1)
                    for i, (ysl, off, q) in enumerate(ops):
                        opi = piece_of_oh[q]
                        ohsl = ohslabs[opi]
                        oho = (q - oh_b[opi]) * 128
                        nc.tensor.matmul(out=psum_s2[:], lhsT=ysl[:, off + F1:off + F],
                                         rhs=ohsl[:, oho:oho + 128],
                                         start=i == 0, stop=i == len(ops) - 1)
                    # merge self term during PSUM -> SBUF copy
                    nc.vector.tensor_tensor(
                        out=sT1q[:, j * 128:(j + 1) * 128], in0=psum_s[:],
                        in1=x1[:, xo:xo + 128], op=mybir.AluOpType.add)
                    nc.vector.tensor_tensor(
                        out=sT2q[0:F2, j * 128:(j + 1) * 128], in0=psum_s2[:],
                        in1=x2[:, xo:xo + 128], op=mybir.AluOpType.add)
                if j == TILES_PER_T - 1:
                    # ---- stage 2 batched: z^T = relu(P_t^T s^T + b)
                    pz_a = pzpool.tile([128, 512], dt, space="PSUM", tag="pz")
                    pz_b = pzpool.tile([128, S - 512], dt, space="PSUM", tag="pz")
                    pc0 = tt * H
                    nc.tensor.matmul(out=pz_a[:], lhsT=P1_sb[:, pc0:pc0 + H],
                                     rhs=sT1q[:, 0:512], start=True, stop=False)
                    nc.tensor.matmul(out=pz_b[:], lhsT=P1_sb[:, pc0:pc0 + H],
                                     rhs=sT1q[:, 512:S], start=True, stop=False)
                    nc.tensor.matmul(out=pz_a[:], lhsT=P2_sb[:, pc0:pc0 + H],
                                     rhs=sT2q[0:F2, 0:512], start=False, stop=True)
                    nc.tensor.matmul(out=pz_b[:], lhsT=P2_sb[:, pc0:pc0 + H],
                                     rhs=sT2q[0:F2, 512:S], start=False, stop=True)
                    zTq = ztpool.tile([128, S], bf, tag="zTq")
                    nc.scalar.activation(out=zTq[:, 0:512], in_=pz_a[:],
                                         func=mybir.ActivationFunctionType.Relu,
                                         bias=projb_sb[:, 0:1])
                    nc.scalar.activation(out=zTq[:, 512:S], in_=pz_b[:],
                                         func=mybir.ActivationFunctionType.Relu,
                                         bias=projb_sb[:, 0:1])
                    # ---- stage 3 batched: lg^T for the whole group
                    if tt % GBATCH == 0:
                        lgb = lgpool.tile([C, GBATCH * S], dt, tag="lgb")
                    lo = (tt % GBATCH) * S
                    psum_lg = pzpool.tile([C, 512], dt, space="PSUM", tag="pz")
                    nc.tensor.matmul(out=psum_lg[:], lhsT=clsw_sb[:],
                                     rhs=zTq[:, 0:512], start=True, stop=True)
                    nc.scalar.copy(out=lgb[:, lo:lo + 512], in_=psum_lg[:])
                    psum_lg2 = pzpool.tile([C, 512], dt, space="PSUM", tag="pz")
                    nc.tensor.matmul(out=psum_lg2[:, 0:S - 512], lhsT=clsw_sb[:],
                                     rhs=zTq[:, 512:S], start=True, stop=True)
                    nc.scalar.copy(out=lgb[:, lo + 512:lo + S],
                                   in_=psum_lg2[:, 0:S - 512])
                    if tt % GBATCH == GBATCH - 1 or tt == T - 1:
                        b0 = (tt - tt % GBATCH) * S
                        nc.sync.dma_start(
                            out=lgT_d[:, b0:b0 + (tt % GBATCH) * S + S],
                            in_=lgb[:, 0:(tt % GBATCH) * S + S])
    nc.compile()
    return nc


def kernel(**inputs):
    from concourse.bass_utils import run_bass_kernel_spmd

    np_inputs = {k: np.asarray(v) for k, v in inputs.items()}
    per_core, orig_of, K = _host_prep(**np_inputs)

    if K not in _cache:
        _cache[K] = _build(K)
    nc = _cache[K]

    res = run_bass_kernel_spmd(nc, per_core, list(range(NCORES)))

    cls_b = np_inputs["cls_b"].astype(np.float32)
    logits = np.zeros((N, C), np.float32)
    for c in range(NCORES):
        ids = orig_of[c]
        valid = ids >= 0
        lgT = res.results[c]["lgT"]                    # [C, NPAD]
        logits[ids[valid]] = lgT.T[valid]
    logits += cls_b
    return logits
